# revision 19
# baseline (speedup 1.0000x reference)
"""Trainium2 Bass kernel for nn_Attention: B=8, S=2048, D=1024 single-head
attention with QKV projections, softmax(QK^T/sqrt(S)), and context = P @ V.

Sharding: pure data-parallel over batch — one batch element per NeuronCore,
weights replicated, zero collectives.

Per-core layout strategy (all matmuls bf16 operands, fp32 PSUM accumulate):
  The Q/K projections are algebraically fused: qk[i,j] = x_i G x_j^T + a_i + b_j
  with G = Wq^T Wk (host-precomputed), and the per-row a_i cancels in softmax.
  xT  [D, S]  (host pre-transposed)                      x^T
  h^T [D, S] = G-tile-stationary matmuls over x^T        (h = x G^T)
  beta[j] = x_j . (Wk^T bq)  via N=1 matmuls sharing V's stationary x-tiles
  V   [S, D]  natural layout (bias folded out: sum_j P[i,j] == 1, added host-side)
  A = qk^T [j, i] = h^T-stationary @ x^T-moving          scores, transposed
  E = exp(A/sqrt(S) + beta/sqrt(S))  (bf16, ScalarE, per-partition bias)
  den[i] = sum_j E[j, i]  via N=1 matmuls with a ones column (shares lhsT with ctx)
  ctx[i, d] = (sum_j E[j,i] V[j,d]) * (1/den[i])         E-tile-stationary matmuls,
                                                         per-partition DVE normalize
  P^T[j, i] = E[j,i] * recip[i]   recip row replicated across partitions via a
                                  K=1 outer-product matmul, DVE multiply
"""

import numpy as np
import ml_dtypes

B, S, D = 8, 2048, 1024
P = 128
NCORES = 8
DT = D // P            # 8 d-tiles
ST = S // P            # 16 s-tiles
SC = S // 512          # 4 s-chunks
OC = D // 512          # 2 o-chunks
INV_SCALE = float(1.0 / np.sqrt(np.float32(S)))

_CACHE = {}


def _build():
    import concourse.bass as bass
    import concourse.tile as tile
    from concourse import bacc, mybir

    f32 = mybir.dt.float32
    bf16 = mybir.dt.bfloat16
    Act = mybir.ActivationFunctionType

    nc = bacc.Bacc("TRN2", target_bir_lowering=False, debug=False,
                   enable_asserts=False, num_devices=NCORES)

    xT_d = nc.dram_tensor("xt", [P, DT, S], bf16, kind="ExternalInput")
    wg_d = nc.dram_tensor("wg", [P, DT, D], bf16, kind="ExternalInput")
    wv_d = nc.dram_tensor("wv", [P, DT, D], bf16, kind="ExternalInput")
    c2_d = nc.dram_tensor("c2", [P, DT], bf16, kind="ExternalInput")
    ctx_d = nc.dram_tensor("ctx", [S, D], f32, kind="ExternalOutput")
    pt_d = nc.dram_tensor("pt", [S, S], bf16, kind="ExternalOutput")

    with tile.TileContext(nc) as tc:
        from contextlib import ExitStack
        with ExitStack() as es:
            const = es.enter_context(tc.tile_pool(name="const", bufs=1))
            qkv = es.enter_context(tc.tile_pool(name="qkv", bufs=1))

            c2_sb = const.tile([P, DT], bf16)
            ones_col = const.tile([P, 1], bf16)
            ones_row = const.tile([1, P], f32)
            nc.vector.memset(ones_col[:], 1.0)
            nc.vector.memset(ones_row[:], 1.0)

            xt_sb = qkv.tile([P, DT, S], bf16)
            gt_sb = qkv.tile([P, DT, S], bf16)
            v_sb = qkv.tile([P, ST, D], bf16)
            b_sb = qkv.tile([P, ST], f32)

            # ---------------- phase 1: QKV projections ----------------
            with tc.tile_pool(name="xw", bufs=1) as xw, \
                 tc.tile_pool(name="ps1", bufs=3, space="PSUM") as ps1:
                wg_sb = xw.tile([P, DT, D], bf16)
                wv_sb = xw.tile([P, DT, D], bf16)
                # Few big DMAs (HWDGE trigger cost dominates small ones), with
                # the first o-tile of wg + first s-chunk of xt leading so the
                # first compute groups start ASAP; wv follows (not needed
                # until the V projection runs)
                nc.sync.dma_start(wg_sb[:, :, 0:128], wg_d.ap()[:, :, 0:128])
                nc.sync.dma_start(xt_sb[:, :, 0:512], xT_d.ap()[:, :, 0:512])
                nc.sync.dma_start(wg_sb[:, :, 128:512],
                                  wg_d.ap()[:, :, 128:512])
                nc.sync.dma_start(c2_sb[:], c2_d.ap())
                nc.sync.dma_start(wg_sb[:, :, 512:1024],
                                  wg_d.ap()[:, :, 512:1024])
                nc.sync.dma_start(xt_sb[:, :, 512:1024],
                                  xT_d.ap()[:, :, 512:1024])
                nc.sync.dma_start(xt_sb[:, :, 1024:2048],
                                  xT_d.ap()[:, :, 1024:2048])
                for dt in range(DT):
                    nc.sync.dma_start(wv_sb[:, dt, :], wv_d.ap()[:, dt, :])

                # h^T: [a-tile, s] = G-tile-stationary, xT moving
                for sc in range(SC):
                    for ot in range(DT):
                        ps = ps1.tile([P, 512], f32, tag="mm")
                        for dt in range(DT):
                            nc.tensor.matmul(
                                ps[:],
                                lhsT=wg_sb[:, dt, ot * P:(ot + 1) * P],
                                rhs=xt_sb[:, dt, sc * 512:(sc + 1) * 512],
                                start=(dt == 0), stop=(dt == DT - 1))
                        nc.scalar.copy(
                            gt_sb[:, ot, sc * 512:(sc + 1) * 512], ps[:])

                # V: natural [s-tile, o] = xT-tile-stationary, W moving;
                # beta's N=1 matmuls ride along on the same stationary x-tiles
                for st in range(ST):
                    ps0 = ps1.tile([P, 512], f32, tag="mm")
                    ps1b = ps1.tile([P, 512], f32, tag="mm")
                    psb = ps1.tile([P, 1], f32, tag="beta", bufs=2)
                    for dt in range(DT):
                        st0, sp = (dt == 0), (dt == DT - 1)
                        nc.tensor.matmul(
                            ps0[:], lhsT=xt_sb[:, dt, st * P:(st + 1) * P],
                            rhs=wv_sb[:, dt, 0:512], start=st0, stop=sp)
                        nc.tensor.matmul(
                            ps1b[:], lhsT=xt_sb[:, dt, st * P:(st + 1) * P],
                            rhs=wv_sb[:, dt, 512:1024], start=st0, stop=sp)
                        nc.tensor.matmul(
                            psb[:], lhsT=xt_sb[:, dt, st * P:(st + 1) * P],
                            rhs=c2_sb[:, dt:dt + 1], start=st0, stop=sp)
                    nc.scalar.copy(v_sb[:, st, 0:512], ps0[:])
                    nc.scalar.copy(v_sb[:, st, 512:1024], ps1b[:])
                    nc.scalar.mul(b_sb[:, st:st + 1], psb[:], INV_SCALE)

            # ---------------- phase 2+3, blocked over i-chunks ----------------
            with tc.tile_pool(name="ework", bufs=2) as ework, \
                 tc.tile_pool(name="stage", bufs=3) as stage, \
                 tc.tile_pool(name="ps2", bufs=1, space="PSUM") as ps2:
                for ic in range(SC):
                    e_chunk = ework.tile([P, ST, 512], bf16, tag="E")
                    recip_c = ework.tile([P, SC], f32, tag="recip")

                    # scores qk^T for i-chunk, exp -> E (bf16)
                    for jt in range(ST):
                        psa = ps2.tile([P, 512], f32, tag="A", bufs=2)
                        for ot in range(DT):
                            nc.tensor.matmul(
                                psa[:],
                                lhsT=gt_sb[:, ot, jt * P:(jt + 1) * P],
                                rhs=xt_sb[:, ot, ic * 512:(ic + 1) * 512],
                                start=(ot == 0), stop=(ot == DT - 1))
                        nc.scalar.activation(e_chunk[:, jt, :], psa[:],
                                             Act.Exp, scale=INV_SCALE,
                                             bias=b_sb[:, jt:jt + 1])

                    # denominators first (N=1 matmuls into one shared bank) so
                    # the P^T normalize/writeback overlaps the context matmuls
                    psd4 = ps2.tile([P, SC], f32, tag="den", bufs=1)
                    rrow = stage.tile([1, 512], f32, tag="rrow", bufs=2)
                    for t4 in range(4):
                        for jt in range(ST):
                            nc.tensor.matmul(
                                psd4[:, t4:t4 + 1],
                                lhsT=e_chunk[:, jt, t4 * P:(t4 + 1) * P],
                                rhs=ones_col[:],
                                start=(jt == 0), stop=(jt == ST - 1))
                    for t4 in range(4):
                        nc.vector.reciprocal(recip_c[:, t4:t4 + 1],
                                             psd4[:, t4:t4 + 1])
                        nc.sync.dma_start(rrow[0:1, t4 * P:(t4 + 1) * P],
                                          recip_c[:, t4:t4 + 1])

                    def ctx_group(t4):
                        it = 4 * ic + t4
                        ps0 = ps2.tile([P, 512], f32, tag="mm", bufs=4)
                        psn = ps2.tile([P, 512], f32, tag="mm", bufs=4)
                        for jt in range(ST):
                            lw = e_chunk[:, jt, t4 * P:(t4 + 1) * P]
                            st0, sp = (jt == 0), (jt == ST - 1)
                            nc.tensor.matmul(ps0[:], lhsT=lw,
                                             rhs=v_sb[:, jt, 0:512],
                                             start=st0, stop=sp)
                            nc.tensor.matmul(psn[:], lhsT=lw,
                                             rhs=v_sb[:, jt, 512:1024],
                                             start=st0, stop=sp)
                        for half, psh in ((0, ps0), (1, psn)):
                            ct = stage.tile([P, 512], f32, tag="ctx")
                            nc.vector.tensor_scalar_mul(ct[:], psh[:],
                                                        recip_c[:, t4:t4 + 1])
                            nc.sync.dma_start(
                                ctx_d.ap()[it * P:(it + 1) * P,
                                           half * 512:(half + 1) * 512], ct[:])

                    def pt_write(jts):
                        # P^T normalize + writeback; interleaved between ctx
                        # groups so DVE keeps freeing ctx psum slots promptly
                        for jt in jts:
                            ptt = stage.tile([P, 512], bf16, tag="pt")
                            nc.vector.tensor_mul(ptt[:], e_chunk[:, jt, :],
                                                 psr[:])
                            nc.sync.dma_start(
                                pt_d.ap()[jt * P:(jt + 1) * P,
                                          ic * 512:(ic + 1) * 512], ptt[:])

                    ctx_group(0)
                    # recip row replicated across partitions (K=1 outer product)
                    psr = ps2.tile([P, 512], f32, tag="rr", bufs=1)
                    nc.tensor.matmul(psr[:], lhsT=ones_row[:], rhs=rrow[:],
                                     start=True, stop=True)
                    if ic < SC - 1:
                        ctx_group(1)
                        pt_write(range(0, 5))
                        ctx_group(2)
                        pt_write(range(5, 10))
                        ctx_group(3)
                        pt_write(range(10, 16))
                    else:
                        # final block: finish all P^T writes before the last
                        # ctx group so the kernel doesn't trail on DVE/DMA
                        ctx_group(1)
                        pt_write(range(0, 8))
                        ctx_group(2)
                        pt_write(range(8, 16))
                        ctx_group(3)

    nc.compile()
    return nc


def _get_nc():
    if "nc" not in _CACHE:
        _CACHE["nc"] = _build()
    return _CACHE["nc"]


def _prep_inputs(out_hid, QW_w, QW_b, KW_w, KW_b, VW_w, VW_b):
    bf = ml_dtypes.bfloat16
    out_hid = np.asarray(out_hid)
    QW_w, QW_b = np.asarray(QW_w, np.float32), np.asarray(QW_b, np.float32)
    KW_w = np.asarray(KW_w, np.float32)
    VW_w = np.asarray(VW_w)
    # fused score weight: qk[i,j] = x_i G x_j^T + (terms constant per i, which
    # cancel in softmax) + x_j . c2;  G = Wq^T Wk, c2 = Wk^T bq
    G = QW_w.T @ KW_w
    c2 = KW_w.T @ QW_b
    # weights/biases replicated across cores
    def w_arr(W):  # [p, dt, o] = W[o, dt*128+p]
        return np.ascontiguousarray(
            W.T.reshape(DT, P, D).transpose(1, 0, 2)).astype(bf)

    shared = {
        "wg": w_arr(G), "wv": w_arr(VW_w),
        "c2": np.ascontiguousarray(c2.reshape(DT, P).T).astype(bf),
    }
    in_maps = []
    for b in range(B):
        x = out_hid[b]  # [S, D]
        xt = np.ascontiguousarray(
            x.reshape(S, DT, P).transpose(2, 1, 0)).astype(bf)  # [p, dt, s]
        m = dict(shared)
        m["xt"] = xt
        in_maps.append(m)
    return in_maps


def kernel(out_hid, QW_w, QW_b, KW_w, KW_b, VW_w, VW_b):
    from concourse.bass_utils import run_bass_kernel_spmd

    nc = _get_nc()
    in_maps = _prep_inputs(out_hid, QW_w, QW_b, KW_w, KW_b, VW_w, VW_b)
    res = run_bass_kernel_spmd(nc, in_maps, core_ids=list(range(NCORES)))

    vb = np.asarray(VW_b, np.float32)
    ctx = np.empty((B, S, D), np.float32)
    for c in range(B):
        ctx[c] = res.results[c]["ctx"] + vb[None, :]
    pt1 = np.asarray(res.results[1]["pt"]).astype(np.float32)
    attn_w1 = np.ascontiguousarray(pt1.T)  # P[i, j] = P^T[j, i]
    return ctx, attn_w1


# revision 20
# speedup vs baseline: 1.0052x; 1.0052x over previous
"""Trainium2 Bass kernel for nn_Attention: B=8, S=2048, D=1024 single-head
attention with QKV projections, softmax(QK^T/sqrt(S)), and context = P @ V.

Sharding: pure data-parallel over batch — one batch element per NeuronCore,
weights replicated, zero collectives.

Per-core layout strategy (all matmuls bf16 operands, fp32 PSUM accumulate):
  The Q/K projections are algebraically fused: qk[i,j] = x_i G x_j^T + a_i + b_j
  with G = Wq^T Wk (host-precomputed), and the per-row a_i cancels in softmax.
  xT  [D, S]  (host pre-transposed)                      x^T
  h^T [D, S] = G-tile-stationary matmuls over x^T        (h = x G^T)
  beta[j] = x_j . (Wk^T bq)  via N=1 matmuls sharing V's stationary x-tiles
  V   [S, D]  natural layout (bias folded out: sum_j P[i,j] == 1, added host-side)
  A = qk^T [j, i] = h^T-stationary @ x^T-moving          scores, transposed
  E = exp(A/sqrt(S) + beta/sqrt(S))  (bf16, ScalarE, per-partition bias)
  den[i] = sum_j E[j, i]  via N=1 matmuls with a ones column (shares lhsT with ctx)
  ctx[i, d] = (sum_j E[j,i] V[j,d]) * (1/den[i])         E-tile-stationary matmuls,
                                                         per-partition DVE normalize
  P^T[j, i] = E[j,i] * recip[i]   recip row replicated across partitions via a
                                  K=1 outer-product matmul, DVE multiply
"""

import numpy as np
import ml_dtypes

B, S, D = 8, 2048, 1024
P = 128
NCORES = 8
DT = D // P            # 8 d-tiles
ST = S // P            # 16 s-tiles
SC = S // 512          # 4 s-chunks
OC = D // 512          # 2 o-chunks
INV_SCALE = float(1.0 / np.sqrt(np.float32(S)))

_CACHE = {}


def _build():
    import concourse.bass as bass
    import concourse.tile as tile
    from concourse import bacc, mybir

    f32 = mybir.dt.float32
    bf16 = mybir.dt.bfloat16
    Act = mybir.ActivationFunctionType

    nc = bacc.Bacc("TRN2", target_bir_lowering=False, debug=False,
                   enable_asserts=False, num_devices=NCORES)

    xT_d = nc.dram_tensor("xt", [P, DT, S], bf16, kind="ExternalInput")
    wg_d = nc.dram_tensor("wg", [P, DT, D], bf16, kind="ExternalInput")
    wv_d = nc.dram_tensor("wv", [P, DT, D], bf16, kind="ExternalInput")
    c2_d = nc.dram_tensor("c2", [P, DT], bf16, kind="ExternalInput")
    ctx_d = nc.dram_tensor("ctx", [S, D], f32, kind="ExternalOutput")
    pt_d = nc.dram_tensor("pt", [S, S], bf16, kind="ExternalOutput")

    with tile.TileContext(nc) as tc:
        from contextlib import ExitStack
        with ExitStack() as es:
            const = es.enter_context(tc.tile_pool(name="const", bufs=1))
            qkv = es.enter_context(tc.tile_pool(name="qkv", bufs=1))
            psp = es.enter_context(tc.tile_pool(name="psp", bufs=1,
                                                space="PSUM"))

            c2_sb = const.tile([P, DT], bf16)
            ones_col = const.tile([P, 1], bf16)
            ones_row = const.tile([1, P], f32)
            nc.vector.memset(ones_col[:], 1.0)
            nc.vector.memset(ones_row[:], 1.0)

            xt_sb = qkv.tile([P, DT, S], bf16)
            gt_sb = qkv.tile([P, DT, S], bf16)
            v_sb = qkv.tile([P, ST, D], bf16)
            b_sb = qkv.tile([P, ST], f32)

            # ---------------- phase 1: QKV projections ----------------
            with tc.tile_pool(name="xw", bufs=1) as xw:
                wg_sb = xw.tile([P, DT, D], bf16)
                wv_sb = xw.tile([P, DT, D], bf16)
                # Few big DMAs (HWDGE trigger cost dominates small ones), with
                # the first o-tile of wg + first s-chunk of xt leading so the
                # first compute groups start ASAP; wv follows (not needed
                # until the V projection runs)
                nc.sync.dma_start(wg_sb[:, :, 0:128], wg_d.ap()[:, :, 0:128])
                nc.sync.dma_start(xt_sb[:, :, 0:512], xT_d.ap()[:, :, 0:512])
                nc.sync.dma_start(wg_sb[:, :, 128:512],
                                  wg_d.ap()[:, :, 128:512])
                nc.sync.dma_start(c2_sb[:], c2_d.ap())
                nc.sync.dma_start(wg_sb[:, :, 512:1024],
                                  wg_d.ap()[:, :, 512:1024])
                nc.sync.dma_start(xt_sb[:, :, 512:1024],
                                  xT_d.ap()[:, :, 512:1024])
                nc.sync.dma_start(xt_sb[:, :, 1024:2048],
                                  xT_d.ap()[:, :, 1024:2048])
                for dt in range(DT):
                    nc.sync.dma_start(wv_sb[:, dt, :], wv_d.ap()[:, dt, :])

                # h^T: [a-tile, s] = G-tile-stationary, xT moving
                for sc in range(SC):
                    for ot in range(DT):
                        ps = psp.tile([P, 512], f32, tag="mm", bufs=6)
                        for dt in range(DT):
                            nc.tensor.matmul(
                                ps[:],
                                lhsT=wg_sb[:, dt, ot * P:(ot + 1) * P],
                                rhs=xt_sb[:, dt, sc * 512:(sc + 1) * 512],
                                start=(dt == 0), stop=(dt == DT - 1))
                        nc.scalar.copy(
                            gt_sb[:, ot, sc * 512:(sc + 1) * 512], ps[:])

                # V: natural [s-tile, o] = xT-tile-stationary, W moving;
                # beta's N=1 matmuls ride along on the same stationary x-tiles
                for st in range(ST):
                    ps0 = psp.tile([P, 512], f32, tag="mm", bufs=6)
                    ps1b = psp.tile([P, 512], f32, tag="mm", bufs=6)
                    psb = psp.tile([P, 512], f32, tag="aux", bufs=2)
                    for dt in range(DT):
                        st0, sp = (dt == 0), (dt == DT - 1)
                        nc.tensor.matmul(
                            ps0[:], lhsT=xt_sb[:, dt, st * P:(st + 1) * P],
                            rhs=wv_sb[:, dt, 0:512], start=st0, stop=sp)
                        nc.tensor.matmul(
                            ps1b[:], lhsT=xt_sb[:, dt, st * P:(st + 1) * P],
                            rhs=wv_sb[:, dt, 512:1024], start=st0, stop=sp)
                        nc.tensor.matmul(
                            psb[:, 0:1],
                            lhsT=xt_sb[:, dt, st * P:(st + 1) * P],
                            rhs=c2_sb[:, dt:dt + 1], start=st0, stop=sp)
                    nc.scalar.copy(v_sb[:, st, 0:512], ps0[:])
                    nc.scalar.copy(v_sb[:, st, 512:1024], ps1b[:])
                    nc.scalar.mul(b_sb[:, st:st + 1], psb[:, 0:1], INV_SCALE)

            # ---------------- phase 2+3, blocked over i-chunks ----------------
            with tc.tile_pool(name="ework", bufs=2) as ework, \
                 tc.tile_pool(name="stage", bufs=3) as stage:
                for ic in range(SC):
                    e_chunk = ework.tile([P, ST, 512], bf16, tag="E")
                    recip_c = ework.tile([P, SC], f32, tag="recip")

                    # scores qk^T for i-chunk, exp -> E (bf16)
                    for jt in range(ST):
                        psa = psp.tile([P, 512], f32, tag="mm", bufs=6)
                        for ot in range(DT):
                            nc.tensor.matmul(
                                psa[:],
                                lhsT=gt_sb[:, ot, jt * P:(jt + 1) * P],
                                rhs=xt_sb[:, ot, ic * 512:(ic + 1) * 512],
                                start=(ot == 0), stop=(ot == DT - 1))
                        nc.scalar.activation(e_chunk[:, jt, :], psa[:],
                                             Act.Exp, scale=INV_SCALE,
                                             bias=b_sb[:, jt:jt + 1])

                    # denominators first (N=1 matmuls into one shared bank) so
                    # the P^T normalize/writeback overlaps the context matmuls
                    psd4 = psp.tile([P, 512], f32, tag="aux", bufs=2)
                    rrow = stage.tile([1, 512], f32, tag="rrow", bufs=2)
                    for t4 in range(4):
                        for jt in range(ST):
                            nc.tensor.matmul(
                                psd4[:, t4:t4 + 1],
                                lhsT=e_chunk[:, jt, t4 * P:(t4 + 1) * P],
                                rhs=ones_col[:],
                                start=(jt == 0), stop=(jt == ST - 1))
                    for t4 in range(4):
                        nc.vector.reciprocal(recip_c[:, t4:t4 + 1],
                                             psd4[:, t4:t4 + 1])
                        nc.sync.dma_start(rrow[0:1, t4 * P:(t4 + 1) * P],
                                          recip_c[:, t4:t4 + 1])

                    def ctx_group(t4):
                        it = 4 * ic + t4
                        ps0 = psp.tile([P, 512], f32, tag="mm", bufs=6)
                        psn = psp.tile([P, 512], f32, tag="mm", bufs=6)
                        for jt in range(ST):
                            lw = e_chunk[:, jt, t4 * P:(t4 + 1) * P]
                            st0, sp = (jt == 0), (jt == ST - 1)
                            nc.tensor.matmul(ps0[:], lhsT=lw,
                                             rhs=v_sb[:, jt, 0:512],
                                             start=st0, stop=sp)
                            nc.tensor.matmul(psn[:], lhsT=lw,
                                             rhs=v_sb[:, jt, 512:1024],
                                             start=st0, stop=sp)
                        for half, psh in ((0, ps0), (1, psn)):
                            ct = stage.tile([P, 512], f32, tag="ctx")
                            nc.vector.tensor_scalar_mul(ct[:], psh[:],
                                                        recip_c[:, t4:t4 + 1])
                            nc.sync.dma_start(
                                ctx_d.ap()[it * P:(it + 1) * P,
                                           half * 512:(half + 1) * 512], ct[:])

                    def pt_write(jts):
                        # P^T normalize + writeback; interleaved between ctx
                        # groups so DVE keeps freeing ctx psum slots promptly
                        for jt in jts:
                            ptt = stage.tile([P, 512], bf16, tag="pt")
                            nc.vector.tensor_mul(ptt[:], e_chunk[:, jt, :],
                                                 psr[:])
                            nc.sync.dma_start(
                                pt_d.ap()[jt * P:(jt + 1) * P,
                                          ic * 512:(ic + 1) * 512], ptt[:])

                    ctx_group(0)
                    # recip row replicated across partitions (K=1 outer product)
                    psr = psp.tile([P, 512], f32, tag="aux", bufs=2)
                    nc.tensor.matmul(psr[:], lhsT=ones_row[:], rhs=rrow[:],
                                     start=True, stop=True)
                    if ic < SC - 1:
                        ctx_group(1)
                        pt_write(range(0, 5))
                        ctx_group(2)
                        pt_write(range(5, 10))
                        ctx_group(3)
                        pt_write(range(10, 16))
                    else:
                        # final block: finish all P^T writes before the last
                        # ctx group so the kernel doesn't trail on DVE/DMA
                        ctx_group(1)
                        pt_write(range(0, 8))
                        ctx_group(2)
                        pt_write(range(8, 16))
                        ctx_group(3)

    nc.compile()
    return nc


def _get_nc():
    if "nc" not in _CACHE:
        _CACHE["nc"] = _build()
    return _CACHE["nc"]


def _prep_inputs(out_hid, QW_w, QW_b, KW_w, KW_b, VW_w, VW_b):
    bf = ml_dtypes.bfloat16
    out_hid = np.asarray(out_hid)
    QW_w, QW_b = np.asarray(QW_w, np.float32), np.asarray(QW_b, np.float32)
    KW_w = np.asarray(KW_w, np.float32)
    VW_w = np.asarray(VW_w)
    # fused score weight: qk[i,j] = x_i G x_j^T + (terms constant per i, which
    # cancel in softmax) + x_j . c2;  G = Wq^T Wk, c2 = Wk^T bq
    G = QW_w.T @ KW_w
    c2 = KW_w.T @ QW_b
    # weights/biases replicated across cores
    def w_arr(W):  # [p, dt, o] = W[o, dt*128+p]
        return np.ascontiguousarray(
            W.T.reshape(DT, P, D).transpose(1, 0, 2)).astype(bf)

    shared = {
        "wg": w_arr(G), "wv": w_arr(VW_w),
        "c2": np.ascontiguousarray(c2.reshape(DT, P).T).astype(bf),
    }
    in_maps = []
    for b in range(B):
        x = out_hid[b]  # [S, D]
        xt = np.ascontiguousarray(
            x.reshape(S, DT, P).transpose(2, 1, 0)).astype(bf)  # [p, dt, s]
        m = dict(shared)
        m["xt"] = xt
        in_maps.append(m)
    return in_maps


def kernel(out_hid, QW_w, QW_b, KW_w, KW_b, VW_w, VW_b):
    from concourse.bass_utils import run_bass_kernel_spmd

    nc = _get_nc()
    in_maps = _prep_inputs(out_hid, QW_w, QW_b, KW_w, KW_b, VW_w, VW_b)
    res = run_bass_kernel_spmd(nc, in_maps, core_ids=list(range(NCORES)))

    vb = np.asarray(VW_b, np.float32)
    ctx = np.empty((B, S, D), np.float32)
    for c in range(B):
        ctx[c] = res.results[c]["ctx"] + vb[None, :]
    pt1 = np.asarray(res.results[1]["pt"]).astype(np.float32)
    attn_w1 = np.ascontiguousarray(pt1.T)  # P[i, j] = P^T[j, i]
    return ctx, attn_w1
